# revision 20
# baseline (speedup 1.0000x reference)
"""Trainium2 Bass kernel for ContactDiffusion GNN message passing.

out = latent + K_norm @ msg,  K = (D+eps)^(-alpha_ij) * exp(-D/12), row-normalized,
msg = MLP(latent).

v2 design (8 NeuronCores, SPMD, full inputs in / full output out):
 - Host: KD-sort points spatially; each core owns 1024 contiguous sorted rows.
 - Per-core j-block order is ROTATED by the core id so every core's own block
   sits at slots 0..7 of the 64-slot slab loop -> identical SPMD program.
 - Device: one unified Gram pass for all 64 j-tiles ([128 j x 1024 i] via a
   17-feature fp16-split matmul), elementwise chain Ln -> Exp(D/6) on ScalarE
   (single activation table set, patched chooser), (ah_i+ah_j)*l and +D/6 on
   DVE, final Exp -> bf16 K with free row-sum accumulation (accum_out).
 - Diagonal of own slots: Ln bias keeps d2>0; affine_select zeroes the diag;
   row sums for those slots via DVE reduce post-select.
 - Close pairs (d2 < TSTRAG, i != j, any core) suppressed on device via a
   rank-1 indicator feature; exact K added back on host from the device's own
   bf16 msg output. Row sums assembled on host by symmetry (column partials).
 - MLP sharded; msg AllGathered in two halves so contraction of half the
   slots can start ~30us earlier; per-slot msg tiles fetched with indirect
   DMA driven by a per-core index table (realizes the rotation).
"""

import math
import sys
from contextlib import ExitStack

import numpy as np

sys.path.insert(0, "/opt/trn_rl_repo")

import ml_dtypes

import concourse.bass as bass
import concourse.tile as tile
from concourse import bacc, mybir
from concourse.bass_utils import run_bass_kernel_spmd

F32 = mybir.dt.float32
F16 = mybir.dt.float16
BF16 = mybir.dt.bfloat16
I32 = mybir.dt.int32
AF = mybir.ActivationFunctionType
ALU = mybir.AluOpType

NP_BF16 = ml_dtypes.bfloat16

N, DIM, NCORE = 8192, 512, 8
NSH = N // NCORE
NT = N // 128               # 64 j-slots
EPS, LAM = 1e-4, 12.0
TSTRAG = 0.09
SUP = 1e3
B_LN = 2e-3                 # Ln input bias: keeps diag d2 > 0 (gram err <1.2e-3)
GROUP = 8
LN6 = math.log(6.0)

_BUILT = {}


def _patch_act_tables():
    """Force the activation-table chooser to keep Ln/Exp/Square/Copy in the
    combined natural_log_exp set (and Gelu alone in its set) so the slab loop
    runs with zero table reloads. Only narrows choices; emitted set ids still
    index the true act_info.json order."""
    import concourse.hw_specs as hw_specs

    if getattr(hw_specs.get_activation_tables, "_patched_v2", False):
        return
    orig = hw_specs.get_activation_tables
    contested = {AF.Ln, AF.Exp, AF.Square, AF.Copy, AF.Identity, AF.Gelu,
                 AF.MemsetZero}

    def patched(arch):
        tabs = orig(arch)
        out = {}
        for name, s in tabs.items():
            if name == "natural_log_exp_and_others":
                out[name] = set(s) | {AF.Copy, AF.Identity}
            elif name == "gelu_and_others":
                out[name] = (set(s) - contested) | {AF.Gelu}
            else:
                out[name] = set(s) - contested
        return out

    patched._patched_v2 = True
    hw_specs.get_activation_tables = patched
    bacc.get_activation_tables = patched


# ----------------------------------------------------------------------------
# device program
# ----------------------------------------------------------------------------
def build_program(trace_sim=False):
    nsh = NSH
    n_kd = DIM // 128           # 4 contraction k-blocks for MLP
    n_ic = nsh // 128           # 8 i-chunks

    _patch_act_tables()
    nc = bacc.Bacc("TRN2", target_bir_lowering=False, debug=False,
                   num_devices=NCORE)

    featj = nc.dram_tensor("featj", [17, N], F16, kind="ExternalInput").ap()
    feati = nc.dram_tensor("feati", [17, nsh], F16, kind="ExternalInput").ap()
    ahj = nc.dram_tensor("ahj", [128, NT], F32, kind="ExternalInput").ap()
    ahibc = nc.dram_tensor("ahibc", [128, nsh], F16, kind="ExternalInput").ap()
    idxt = nc.dram_tensor("idxt", [128, NT], I32, kind="ExternalInput").ap()
    latT = nc.dram_tensor("latT", [DIM, nsh], F16, kind="ExternalInput").ap()
    w1t = nc.dram_tensor("w1t", [DIM, DIM], F16, kind="ExternalInput").ap()
    w2t = nc.dram_tensor("w2t", [DIM, DIM], F16, kind="ExternalInput").ap()
    b1c = nc.dram_tensor("b1c", [128, n_kd], F32, kind="ExternalInput").ap()
    b2r = nc.dram_tensor("b2r", [1, DIM], F16, kind="ExternalInput").ap()
    onescol = nc.dram_tensor("onescol", [1, 128], F16, kind="ExternalInput").ap()

    num_out = nc.dram_tensor("num", [nsh, DIM], F32, kind="ExternalOutput").ap()
    srow_out = nc.dram_tensor("srow", [128, NT], F32, kind="ExternalOutput").ap()
    msgo_out = nc.dram_tensor("msgo", [nsh, DIM], BF16, kind="ExternalOutput").ap()

    with tile.TileContext(nc, trace_sim=trace_sim) as tc, ExitStack() as ctx:
        pers = ctx.enter_context(tc.tile_pool(name="pers", bufs=1))
        p_big = ctx.enter_context(tc.tile_pool(name="pbig", bufs=2, space="PSUM"))
        p_out = ctx.enter_context(tc.tile_pool(name="pout", bufs=2, space="PSUM"))
        l_pool = ctx.enter_context(tc.tile_pool(name="lp", bufs=3))
        d12_pool = ctx.enter_context(tc.tile_pool(name="d12", bufs=3))
        m_pool = ctx.enter_context(tc.tile_pool(name="mp", bufs=2))
        t_pool = ctx.enter_context(tc.tile_pool(name="tp", bufs=2))
        k_pool = ctx.enter_context(tc.tile_pool(name="kp", bufs=34))
        kraw_pool = ctx.enter_context(tc.tile_pool(name="kraw", bufs=2))
        msg_pool = ctx.enter_context(tc.tile_pool(name="msgp", bufs=34))
        dram = ctx.enter_context(tc.tile_pool(name="dram", bufs=1, space="DRAM"))

        dma = nc.sync.dma_start

        # ---- warmup collective: absorb NEFF launch skew during load phase ----
        warm_sb = pers.tile([8, 8], F32)
        nc.gpsimd.memset(warm_sb[:], 1.0)
        warm_d = dram.tile([8, 8], F32)
        warm_out = dram.tile([64, 8], F32)
        dma(warm_d[:], warm_sb[:])
        nc.gpsimd.collective_compute(
            "AllGather", ALU.bypass,
            ins=[warm_d.opt()], outs=[warm_out.opt()],
            replica_groups=[list(range(NCORE))])

        # ---- persistent SBUF loads ----
        featj_sb = pers.tile([17, N], F16)
        dma(featj_sb[:], featj[:])
        feati_sb = pers.tile([17, nsh], F16)
        dma(feati_sb[:], feati[:])
        ahj_sb = pers.tile([128, NT], F32)
        dma(ahj_sb[:], ahj[:])
        ahibc_sb = pers.tile([128, nsh], F16)
        dma(ahibc_sb[:], ahibc[:])
        idx_sb = pers.tile([128, NT], I32)
        dma(idx_sb[:], idxt[:])
        b1c_sb = pers.tile([128, n_kd], F32)
        dma(b1c_sb[:], b1c[:])
        b2r_sb = pers.tile([1, DIM], F16)
        dma(b2r_sb[:], b2r[:])
        onescol_sb = pers.tile([1, 128], F16)
        dma(onescol_sb[:], onescol[:])
        latT_sb = [pers.tile([128, nsh], F16, tag=f"latT{k}", name=f"latT{k}") for k in range(n_kd)]
        for k in range(n_kd):
            dma(latT_sb[k][:], latT[k * 128:(k + 1) * 128, :])
        w1t_sb = [pers.tile([128, DIM], F16, tag=f"w1t{k}", name=f"w1t{k}") for k in range(n_kd)]
        w2t_sb = [pers.tile([128, DIM], F16, tag=f"w2t{k}", name=f"w2t{k}") for k in range(n_kd)]
        for k in range(n_kd):
            dma(w1t_sb[k][:], w1t[k * 128:(k + 1) * 128, :])
            dma(w2t_sb[k][:], w2t[k * 128:(k + 1) * 128, :])

        acc = pers.tile([128, n_ic * DIM], F32)
        nc.gpsimd.memset(acc[:], 0.0)
        sacc = pers.tile([128, NT], F32)

        bias_b = pers.tile([128, 1], F32)
        nc.gpsimd.memset(bias_b[:], B_LN)
        bias_ln6 = pers.tile([128, 1], F32)
        nc.gpsimd.memset(bias_ln6[:], -LN6)

        msgown_d = dram.tile([nsh, DIM], BF16)
        msgall_d = dram.tile([N, DIM], BF16)

        # ---- phase A: MLP ----
        cw = 512
        hT_sb = [pers.tile([128, nsh], F16, tag=f"hT{k}", name=f"hT{k}") for k in range(n_kd)]
        for mc in range(n_kd):
            ph = p_big.tile([128, nsh], F32, tag="big", name="ph")
            for half in range(nsh // cw):
                hs = slice(half * cw, (half + 1) * cw)
                for kb in range(n_kd):
                    nc.tensor.matmul(
                        ph[:, hs],
                        lhsT=w1t_sb[kb][:, mc * 128:(mc + 1) * 128],
                        rhs=latT_sb[kb][:, hs],
                        start=(kb == 0), stop=(kb == n_kd - 1))
            nc.scalar.activation(hT_sb[mc][:], ph[:], AF.Gelu,
                                 bias=b1c_sb[:, mc:mc + 1], scale=1.0)

        msgown_sb = [pers.tile([128, DIM], BF16, tag=f"mo{ic}", name=f"mo{ic}") for ic in range(n_ic)]

        def emit_msgown(ic):
            pm = p_out.tile([128, DIM], F32, tag="out", name="pm")
            for kb in range(n_kd):
                nc.tensor.matmul(
                    pm[:],
                    lhsT=hT_sb[kb][:, ic * 128:(ic + 1) * 128],
                    rhs=w2t_sb[kb][:],
                    start=(kb == 0), stop=False)
            nc.tensor.matmul(pm[:], lhsT=onescol_sb[:], rhs=b2r_sb[:],
                             start=False, stop=True)
            nc.vector.tensor_copy(msgown_sb[ic][:], pm[:])
            dma(msgown_d[ic * 128:(ic + 1) * 128, :], msgown_sb[ic][:])
            dma(msgo_out[ic * 128:(ic + 1) * 128, :], msgown_sb[ic][:])

        for ic in range(n_ic):
            emit_msgown(ic)
        nc.gpsimd.collective_compute(
            "AllGather", ALU.bypass,
            ins=[msgown_d.opt()], outs=[msgall_d.opt()],
            replica_groups=[list(range(NCORE))])

        # ---- slab loop ---- (own slots first: their contraction needs no AG)
        order = list(range(NT))

        def emit_elementwise(s):
            pd2 = p_big.tile([128, nsh], F32, tag="big", name="pd2")
            for half in range(nsh // cw):
                hs = slice(half * cw, (half + 1) * cw)
                nc.tensor.matmul(pd2[:, hs],
                                 lhsT=featj_sb[:, s * 128:(s + 1) * 128],
                                 rhs=feati_sb[:, hs],
                                 start=True, stop=True)
            l = l_pool.tile([128, nsh], F16)
            nc.scalar.activation(l[:], pd2[:], AF.Ln, bias=bias_b[:, 0:1])
            d12 = d12_pool.tile([128, nsh], F16)
            nc.scalar.activation(d12[:], l[:], AF.Exp, bias=bias_ln6[:, 0:1],
                                 scale=0.5)
            m = m_pool.tile([128, nsh], F16)
            nc.vector.scalar_tensor_tensor(
                m[:], ahibc_sb[:], ahj_sb[:, s:s + 1], l[:],
                op0=ALU.add, op1=ALU.mult)
            t = t_pool.tile([128, nsh], F16)
            nc.vector.tensor_tensor(t[:], m[:], d12[:], op=ALU.add)
            if s < 8:
                kraw = kraw_pool.tile([128, nsh], BF16, tag="kraw", name="kraw")
                nc.scalar.activation(kraw[:], t[:], AF.Exp, scale=-0.5)
                ktile = k_pool.tile([128, nsh], BF16)
                nc.gpsimd.affine_select(
                    ktile[:], kraw[:], pattern=[[1, nsh]],
                    compare_op=ALU.not_equal, fill=0.0,
                    base=-(s * 128), channel_multiplier=-1)
                nc.vector.tensor_reduce(
                    sacc[:, s:s + 1], ktile[:], axis=mybir.AxisListType.X,
                    op=ALU.add)
                return ktile, msgown_sb[s]
            ktile = k_pool.tile([128, nsh], BF16)
            nc.scalar.activation(ktile[:], t[:], AF.Exp, scale=-0.5,
                                 accum_out=sacc[:, s:s + 1])
            mt = msg_pool.tile([128, DIM], BF16)
            nc.gpsimd.indirect_dma_start(
                out=mt[:], out_offset=None, in_=msgall_d[:],
                in_offset=bass.IndirectOffsetOnAxis(ap=idx_sb[:, s:s + 1],
                                                    axis=0))
            return ktile, mt

        def emit_contraction(tiles):
            g = len(tiles)
            for ic in range(n_ic):
                po = p_out.tile([128, DIM], F32, tag="out", name="po")
                for i, (kt, mt) in enumerate(tiles):
                    nc.tensor.matmul(
                        po[:],
                        lhsT=kt[:, ic * 128:(ic + 1) * 128],
                        rhs=mt[:],
                        start=(i == 0), stop=(i == g - 1))
                asl = slice(ic * DIM, (ic + 1) * DIM)
                nc.vector.tensor_tensor(acc[:, asl], acc[:, asl], po[:],
                                        op=ALU.add)

        # software pipeline: contraction of group k runs while group k+3's
        # elementwise streams, so PE never head-of-line-blocks the Grams and
        # has plenty of queued Gram work while the AllGather completes
        DEPTH = 3
        groups = [order[p:p + GROUP] for p in range(0, NT, GROUP)]
        done = []
        for gi, grp in enumerate(groups):
            done.append([emit_elementwise(s) for s in grp])
            if gi >= DEPTH:
                emit_contraction(done[gi - DEPTH])
        for gi in range(len(groups) - DEPTH, len(groups)):
            emit_contraction(done[gi])

        # ---- epilogue ----
        dma(srow_out[:], sacc[:])
        for ic in range(n_ic):
            dma(num_out[ic * 128:(ic + 1) * 128, :],
                acc[:, ic * DIM:(ic + 1) * DIM])

    nc.compile()
    return nc


# ----------------------------------------------------------------------------
# host-side preprocessing
# ----------------------------------------------------------------------------
def _kdsort(coords, nblocks):
    def rec(idx, nb):
        if nb == 1:
            return [idx]
        pts = coords[idx]
        ax = int(np.argmax(pts.max(0) - pts.min(0)))
        order = np.argsort(pts[:, ax], kind="stable")
        half = len(idx) // 2
        return rec(idx[order[:half]], nb // 2) + rec(idx[order[half:]], nb // 2)

    return np.concatenate(rec(np.arange(coords.shape[0]), nblocks))


def _split16(x):
    hi = x.astype(np.float16).astype(np.float32)
    lo = (x - hi).astype(np.float16).astype(np.float32)
    return hi, lo


def kernel(latent, coords, alpha, W1, b1, W2, b2):
    latent = np.asarray(latent, np.float32)
    coords = np.asarray(coords, np.float32)
    alpha = np.asarray(alpha, np.float32)
    W1 = np.asarray(W1, np.float32)
    b1 = np.asarray(b1, np.float32)
    W2 = np.asarray(W2, np.float32)
    b2 = np.asarray(b2, np.float32)

    perm = _kdsort(coords.astype(np.float64), 64)
    cs = coords[perm]
    als = alpha[perm]
    lats = latent[perm]
    c64 = cs.astype(np.float64)

    core_of = np.arange(N) // NSH
    # stragglers: ANY close pair (d2 < TSTRAG, i != j), grouped by i's core
    Jstar = [set() for _ in range(NCORE)]
    Istar = [set() for _ in range(NCORE)]
    d2min = np.empty(N)
    for i0 in range(0, N, 1024):
        blk = cs[i0:i0 + 1024].astype(np.float64)
        d2b = ((blk[:, None, :] - c64[None, :, :]) ** 2).sum(-1)
        d2b[np.arange(1024), np.arange(i0, i0 + 1024)] = np.inf
        d2min[i0:i0 + 1024] = d2b.min(1)
        ii, jj = np.nonzero(d2b < TSTRAG)
        ii = ii + i0
        for a, b in zip(ii, jj):
            c = core_of[a]
            Jstar[c].add(int(b))
            Istar[c].add(int(a - c * NSH))

    r = (c64 ** 2).sum(-1).astype(np.float32)
    a2 = (-2.0 * cs).astype(np.float32)
    chj = [_split16(cs[:, d]) for d in range(3)]
    ahi = [_split16(a2[:, d]) for d in range(3)]
    rj = _split16(r)
    ah = (als / 2.0).astype(np.float32)

    in_maps = []
    for core in range(NCORE):
        blk = slice(core * NSH, (core + 1) * NSH)
        rot = (np.arange(N) + core * NSH) % N   # slot row -> global row
        rows_j, rows_i = [], []
        for d in range(3):
            for (jp, ip) in [(chj[d][0], ahi[d][0]), (chj[d][0], ahi[d][1]),
                             (chj[d][1], ahi[d][0]), (chj[d][1], ahi[d][1])]:
                rows_j.append(jp)
                rows_i.append(ip[blk])
        ones = np.ones(N, np.float32)
        onesi = np.ones(NSH, np.float32)
        rows_j += [rj[0], rj[1]]
        rows_i += [onesi, onesi]
        rows_j += [ones, ones]
        rows_i += [rj[0][blk], rj[1][blk]]
        g = np.zeros(N, np.float32)
        h = np.zeros(NSH, np.float32)
        for j in Jstar[core]:
            g[j] = SUP
        for i in Istar[core]:
            h[i] = SUP
        rows_j += [g]
        rows_i += [h]
        featj = np.stack(rows_j)[:, rot].astype(np.float16)
        feati = np.stack(rows_i).astype(np.float16)

        ahj = ah[rot].reshape(NT, 128).T.copy()              # [128, 64]
        ahibc = np.broadcast_to(ah[blk], (128, NSH)).astype(np.float16).copy()
        # indirect-gather indices into the all-gathered msg (realizes rotation)
        idxt = np.zeros((128, NT), np.int32)
        p = np.arange(128)
        for s in range(8, NT):
            idxt[:, s] = ((core * 8 + s) % NT) * 128 + p
        in_maps.append({
            "featj": featj, "feati": feati,
            "ahj": np.ascontiguousarray(ahj),
            "ahibc": ahibc, "idxt": idxt,
            "latT": lats[blk].T.astype(np.float16).copy(),
            "w1t": W1.T.astype(np.float16).copy(),
            "w2t": W2.T.astype(np.float16).copy(),
            "b1c": b1.reshape(4, 128).T.astype(np.float32).copy(),
            "b2r": b2.reshape(1, DIM).astype(np.float16),
            "onescol": np.ones((1, 128), np.float16),
        })

    if "nc" not in _BUILT:
        _BUILT["nc"] = build_program()
    nc = _BUILT["nc"]
    res = run_bass_kernel_spmd(nc, in_maps, core_ids=list(range(NCORE)))

    num_all = np.zeros((N, DIM), np.float32)
    s_all = np.zeros(N, np.float32)
    msg_dev = np.zeros((N, DIM), np.float32)
    for core in range(NCORE):
        blk = slice(core * NSH, (core + 1) * NSH)
        num_all[blk] = res.results[core]["num"]
        msg_dev[blk] = res.results[core]["msgo"].astype(np.float32)
        rot = (np.arange(N) + core * NSH) % N
        s_all[rot] += res.results[core]["srow"].T.reshape(-1)

    # host fix: exact K over the suppressed straggler grid (diag excluded)
    for core in range(NCORE):
        J = sorted(Jstar[core])
        I = sorted(Istar[core])
        if not J or not I:
            continue
        Ig = np.array(I) + core * NSH
        d2c = ((c64[J][:, None, :] - c64[Ig][None, :, :]) ** 2).sum(-1)
        diag = (np.array(J)[:, None] == Ig[None, :])
        Dc = np.sqrt(d2c)
        aijc = (als[J].astype(np.float64)[:, None]
                + als[Ig].astype(np.float64)[None, :]) * 0.5
        Kc = (Dc + EPS) ** (-aijc) * np.exp(-Dc / LAM)
        Kc[diag] = 0.0
        Kc = Kc.astype(np.float32)
        num_all[Ig] += (Kc.T @ msg_dev[J]).astype(np.float32)
        s_all[np.array(J)] += Kc.sum(1)

    out = lats + num_all / (s_all[:, None] + 1e-8)
    final = np.empty_like(out)
    final[perm] = out
    return final.astype(np.float32)
